# revision 25
# baseline (speedup 1.0000x reference)
"""Trainium2 Bass kernel for BasePropagationGraphPositionalEncoding.

Computes, for each batch element b:
    out[b] = (sum_k coefs[k] * gr_kernel[b, k]) @ x[b] / sum_k coefs[k]
with coefs[k] = (1 - EPS)^k, EPS = 0.01, K = 9.

Sharding: batch dim B=8 across the 8 NeuronCores (data parallel, no
cross-core communication). Each core streams its 37.75 MB of gr_kernel
slabs from HBM at ~405-412 GB/s (the per-engine SDMA ceiling: 16
engines x ~25.6 GB/s on 4 KB row descriptors).

Design (trace-driven; HW exec ~116.4 us vs ~105 us streaming floor):
  - gr_kernel loads are f32 on the HWDGE (sync) ring, 3 DMAs/band of 3
    slabs each (k=6..8 first, matching chain consumption). Band 0 leads
    with a lone k=8 slab so the chain init starts on a 0.5 MB arrival;
    the last band trails with a lone k=0 slab so the final serial tail
    waits on 0.5 MB, not 1.5 MB.
  - The weighted k-sum runs ENTIRELY on VectorE as a serial fused
    scalar_tensor_tensor chain (wk = g_k*c_k + wk, fp16 accumulator):
    init 0.69us + 8 links x 1.22us = ~10.5us/band, co-pacing the
    ~11.5us/band stream. The STT op pays a dual-SBUF-read port penalty
    (1.22us vs 0.69us tensor_scalar) but every attempt to offload slabs
    to ACT scale-casts (interleaved, adds-last, or casts-pipelined-one-
    band-ahead) measured 4-20us SLOWER: Bacc's single-semaphore wait
    merging makes DVE adds transitively wait on unrelated earlier ACT
    queue entries (prior-band staging), and GpSimd/Pool elementwise is
    ~15us/slab (Q7 software path; TensorScalarPtr is ISA-rejected on
    Pool; PSUM APs are ISA-rejected on TensorScalarPtr too).
  - x is loaded f32 on the sync ring AFTER band 0's first two groups:
    its scatter layout ([p, chunk, d], m = c*128+p on partitions) needs
    1024 tiny 256 B descriptors with ~40 ns/descriptor overhead (~6 us
    of SDMA time) that must not sit in front of the chain-init slab.
    One ACT op casts it to fp16. (gpsimd SWDGE cast-during-DMA emitted
    1040 software packets + a 17.7 us ring drain - dead end.)
  - TensorE transposes the summed kernel (8 fp16 [128,128] tiles/band,
    ~110 ns cadence pipelined) via a GPSIMD-built identity, ACT stages
    PSUM->SBUF (the last band splits staging across ACT and DVE to
    halve the end-of-kernel serial tail), and the contraction accumulates
    in PSUM over 8 chunks. The tail stage is emitted one iteration late
    (software pipelining) so no cross-band serialized loop forms.
  - The 1/sum(coefs) normalization rides free on the ScalarE PSUM->SBUF
    output copy (activation scale); per-band output DMAs go on the ACT
    ring so they never block the sync-ring load stream. fp16 keeps the
    total quantization error ~5e-4 << the 2e-2 gate.
"""

import sys

if "/opt/trn_rl_repo" not in sys.path:
    sys.path.insert(0, "/opt/trn_rl_repo")

import numpy as np

import concourse.bass as bass
import concourse.mybir as mybir
from concourse import tile
from concourse.bacc import Bacc
from concourse.masks import make_identity
from concourse.bass_utils import run_bass_kernel_spmd

# Problem shapes (hardcoded per the harness contract).
B, K, N, D = 8, 9, 1024, 64
EPS = 0.01
P = 128          # SBUF partitions
NT = N // P      # 8 row/col tiles of the [N, N] kernel

F32 = mybir.dt.float32
F16 = mybir.dt.float16

R = 1.0 - EPS                                  # coefficient ratio
S = float(sum(R ** k for k in range(K)))       # sum of coefs

# Slab groups per band-DMA, in issue order. The chain consumes groups in
# this order, so put the high-k slabs (chain start) in the first DMA.
# The LAST band gets its low-k group split once more so the final
# serial tail waits on a single 0.5 MB slab instead of 1.5 MB.
GROUPS = [(6, 9), (3, 6), (0, 3)]
GROUPS_LAST = [(6, 9), (3, 6), (1, 3), (0, 1)]

# Who handles each slab. Measured costs (full [128,1024] slab):
#   ACT activation-scale f32->f16: 1.13us
#   DVE tensor_scalar cast 0.69us / fp16 tensor_tensor add 0.68us /
#       fused scalar_tensor_tensor (wk = g*c + wk) 1.22us
#   Pool TENSOR_SCALAR measured 14.9us (!) - Q7 software path, unusable.
# ACT casts {7,5,3,1} into gh tiles (DVE then adds them, 0.68 each);
# DVE takes {6,4,2,0} as fused STT links directly on the chain and
# inits the chain with k=8. Per band: DVE ~8.3us, ACT ~8.5us, both
# ~3us under the ~11.5us stream cadence.
ACT_CASTS = {7, 1}


def build_bass() -> bass.Bass:
    # Bacc (not plain Bass): its compile() runs generate_event_semaphores /
    # move_matmul_waits_to_ldweights, splitting multi-semaphore waits that
    # the 64B ISA instructions (single EVENTS slot) cannot carry.
    nc = Bacc()

    x_d = nc.dram_tensor("x_b", (N, D), F32, kind="ExternalInput")
    g_d = nc.dram_tensor("g_b", (K, N, N), F32, kind="ExternalInput")
    o_d = nc.dram_tensor("out_b", (N, D), F32, kind="ExternalOutput")

    MULT = mybir.AluOpType.mult
    ADD = mybir.AluOpType.add

    with tile.TileContext(nc) as tc:
        with (
            tc.tile_pool(name="consts", bufs=1) as consts,
            tc.tile_pool(name="gr", bufs=4) as gr_pool,
            tc.tile_pool(name="grl", bufs=1) as grl_pool,
            tc.tile_pool(name="wk", bufs=4) as wk_pool,
            tc.tile_pool(name="gh", bufs=3) as gh_pool,
            tc.tile_pool(name="wkt", bufs=2) as wkt_pool,
            tc.tile_pool(name="outp", bufs=2) as out_pool,
            tc.tile_pool(name="ps_t", bufs=4, space=bass.MemorySpace.PSUM) as ps_t,
            tc.tile_pool(name="ps_e", bufs=2, space=bass.MemorySpace.PSUM) as ps_e,
            tc.tile_pool(name="ps_wk", bufs=2, space=bass.MemorySpace.PSUM) as ps_wk,
        ):
            g_r = g_d.rearrange("k (b p) n -> b p k n", p=P)

            def load_band(i):
                """Per-band HWDGE DMAs of slab groups; returns {k: slice}."""
                tiles = {}
                assert i >= 1
                groups = GROUPS_LAST if i == NT - 1 else GROUPS
                for gi, (k0, k1) in enumerate(groups):
                    # distinct tag per (slot, width) so tile shapes stay
                    # consistent within a tag across bands; the last band's
                    # split low-k groups are single-use (bufs=1 pool)
                    tag = f"g{gi}w{k1 - k0}"
                    pool = gr_pool if (k1 - k0) == 3 else grl_pool
                    t = pool.tile([P, (k1 - k0) * N], F32, tag=tag, name=tag)
                    nc.sync.dma_start(t[:], g_r[i, :, k0:k1, :])
                    for k in range(k0, k1):
                        tiles[k] = t[:, (k - k0) * N : (k - k0 + 1) * N]
                return tiles

            # x: f32 on the sync HWDGE ring, issued BEFORE the band loads.
            # Its scatter layout needs 1024 tiny 256 B descriptors; issued
            # first they drain on idle SDMA engines in ~1us. Issued behind
            # the slab flood (or on the ACT ring) the same transfer took
            # ~15us of round-robin interleave with the 4 KB stream packets
            # and stalled the first band's consumers. One ACT op then casts
            # f32 -> bf16. Layout [p, chunk, d]: chunk c is a [128, 64]
            # tile with the contraction index m = c*128 + p on partitions.
            # (NOT gpsimd SWDGE either: the software descriptor path
            # emitted 1040 packets + a 17.7us ring drain that pinned
            # GpSimd until ~27us.)
            # Band 0's k=8 (chain init) and k=6..7 slabs go FIRST on the
            # sync ring so the chain starts on a 0.5 MB arrival (~9.5us).
            # The x scatter-load (1024 tiny 256 B descriptors, ~40ns each
            # of SDMA time) follows them: issued at the very head it
            # delayed the init slab by ~6us (measured: chain init at
            # 16.5us instead of ~10).
            band_tiles = {}
            for k0, k1, tag in ((8, 9, "gs89"), (6, 8, "gs68")):
                t = grl_pool.tile([P, (k1 - k0) * N], F32, tag=tag, name=tag)
                nc.sync.dma_start(t[:], g_r[0, :, k0:k1, :])
                for k in range(k0, k1):
                    band_tiles[k] = t[:, (k - k0) * N : (k - k0 + 1) * N]

            x_f32 = consts.tile([P, NT, D], F32)
            nc.sync.dma_start(x_f32[:], x_d.rearrange("(c p) d -> p c d", p=P))
            x_sb = consts.tile([P, NT, D], F16)
            nc.scalar.activation(
                x_sb[:], x_f32[:], mybir.ActivationFunctionType.Copy, scale=1.0
            )

            for k0, k1, tag in ((3, 6, "g1w3"), (0, 3, "g2w3")):
                t = gr_pool.tile([P, (k1 - k0) * N], F32, tag=tag, name=tag)
                nc.sync.dma_start(t[:], g_r[0, :, k0:k1, :])
                for k in range(k0, k1):
                    band_tiles[k] = t[:, (k - k0) * N : (k - k0 + 1) * N]

            # fp16 identity for TensorE transpose. Built by GPSIMD (memset +
            # affine_select, ~0.5us), then copied through VectorE so the
            # first PE transpose waits on a single semaphore (DVE) -
            # Matmult lowering only supports one sync wait.
            ident_raw = consts.tile([P, P], F16)
            make_identity(nc, ident_raw)
            ident = consts.tile([P, P], F16)
            nc.vector.tensor_copy(ident[:], ident_raw[:])

            # Post-k-sum pipeline for one band: transposes, wkT staging,
            # contraction, output. Emitted one iteration LATE (software
            # pipelining) so no cross-band serialized loop forms.
            def emit_tail(i, wk):
                wkT_sb = wkt_pool.tile([P, NT, P], F16, name="wkT_sb")
                for c in range(NT):
                    wkT_ps = ps_t.tile([P, P], F16, name="wkT_ps")
                    nc.tensor.transpose(wkT_ps[:], wk[:, c * P : (c + 1) * P], ident[:])
                    if i == NT - 1 and c % 2 == 1:
                        # last band's staging IS the serial tail: odd
                        # chunks ride DVE (idle after its chain) so both
                        # halves move in parallel.
                        nc.vector.tensor_copy(wkT_sb[:, c, :], wkT_ps[:])
                    else:
                        nc.scalar.copy(wkT_sb[:, c, :], wkT_ps[:])

                # emb[band i] = sum_c wk_tile(i,c) @ x_chunk(c), accumulated
                # in PSUM (fp32) over the 8 contraction chunks.
                emb_ps = ps_e.tile([P, D], F32, name="emb_ps")
                for c in range(NT):
                    nc.tensor.matmul(
                        emb_ps[:],
                        wkT_sb[:, c, :],
                        x_sb[:, c, :],
                        start=(c == 0),
                        stop=(c == NT - 1),
                    )

                # PSUM -> SBUF with the 1/S normalization folded into the
                # ACT copy's free scale.
                o_sb = out_pool.tile([P, D], F32, name="o_sb")
                nc.scalar.activation(
                    o_sb[:], emb_ps[:], mybir.ActivationFunctionType.Copy,
                    scale=1.0 / S,
                )
                # Output DMA on the ACT HWDGE ring, NOT sync: an out-DMA on
                # the sync FIFO would block the f32 slab loads queued behind
                # it until this band's whole pipeline finishes.
                nc.scalar.dma_start(o_d[i * P : (i + 1) * P, :], o_sb[:])

            pending = None
            for i in range(NT):
                g_ts = band_tiles
                if i + 1 < NT:
                    band_tiles = load_band(i + 1)

                # Weighted k-sum, wk = sum_k r^k * g_k, accumulated in fp16.
                # The scale+cast of each slab is spread across ACT / Pool /
                # DVE per the *_CASTS tables (the serial add chain stays on
                # DVE: fp16 tensor_tensor ADD, 0.68us). k=8 is the DVE chain
                # init; k=0 rides a fused DVE scalar_tensor_tensor at the
                # end (1.22us, cheaper than a separate cast+add there).
                # All-DVE serial chain, accumulator in SBUF. (PSUM
                # accumulator was tried to dodge the dual-SBUF-read port
                # penalty on scalar_tensor_tensor, but the core-v3 ISA
                # rejects PSUM APs on TensorScalarPtr.)
                wk = wk_pool.tile([P, N], F16, name="wk")
                nc.vector.tensor_scalar_mul(wk[:], g_ts[8], R ** 8)
                for k in (7, 6, 5, 4, 3, 2, 1, 0):
                    nc.vector.scalar_tensor_tensor(
                        wk[:], g_ts[k], R ** k, wk[:], MULT, ADD,
                    )

                if pending is not None:
                    emit_tail(*pending)
                pending = (i, wk)

            emit_tail(*pending)

    nc.compile()
    return nc


_NC = None


def _get_nc() -> bass.Bass:
    global _NC
    if _NC is None:
        _NC = build_bass()
    return _NC


def run(x: np.ndarray, gr_kernel: np.ndarray, **spmd_kwargs):
    """Run the SPMD kernel on cores 0-7; returns BassKernelResults."""
    nc = _get_nc()
    in_maps = [
        {
            "x_b": np.ascontiguousarray(x[b], dtype=np.float32),
            "g_b": np.ascontiguousarray(gr_kernel[b], dtype=np.float32),
        }
        for b in range(B)
    ]
    return run_bass_kernel_spmd(nc, in_maps, core_ids=list(range(B)), **spmd_kwargs)


def kernel(x: np.ndarray, gr_kernel: np.ndarray) -> np.ndarray:
    res = run(np.asarray(x), np.asarray(gr_kernel))
    out = np.stack([res.results[b]["out_b"] for b in range(B)], axis=0)
    return out.astype(np.float32, copy=False)


if __name__ == "__main__":
    rng = np.random.default_rng(0)
    x = rng.standard_normal((B, N, D), dtype=np.float32)
    g = rng.standard_normal((B, K, N, N), dtype=np.float32)
    out = kernel(x, g)
    coefs = (1.0 - EPS) ** np.arange(K)
    wk = np.einsum("k,bknm->bnm", coefs, g)
    ref = np.matmul(wk, x) / coefs.sum()
    err = np.linalg.norm(out - ref) / np.linalg.norm(ref)
    print("self-check rel err:", err)


# revision 26
# speedup vs baseline: 1.0329x; 1.0329x over previous
"""Trainium2 Bass kernel for BasePropagationGraphPositionalEncoding.

Computes, for each batch element b:
    out[b] = (sum_k coefs[k] * gr_kernel[b, k]) @ x[b] / sum_k coefs[k]
with coefs[k] = (1 - EPS)^k, EPS = 0.01, K = 9.

Sharding: batch dim B=8 across the 8 NeuronCores (data parallel, no
cross-core communication). Each core streams its 37.75 MB of gr_kernel
slabs from HBM at ~405-412 GB/s (the per-engine SDMA ceiling: 16
engines x ~25.6 GB/s on 4 KB row descriptors).

Design (trace-driven; HW exec ~116.4 us vs ~105 us streaming floor):
  - gr_kernel loads are f32 on the HWDGE (sync) ring, 3 DMAs/band of 3
    slabs each (k=6..8 first, matching chain consumption). Band 0 leads
    with a lone k=8 slab so the chain init starts on a 0.5 MB arrival;
    the last band trails with a lone k=0 slab so the final serial tail
    waits on 0.5 MB, not 1.5 MB.
  - The weighted k-sum runs ENTIRELY on VectorE as a serial fused
    scalar_tensor_tensor chain (wk = g_k*c_k + wk, fp16 accumulator):
    init 0.69us + 8 links x 1.22us = ~10.5us/band, co-pacing the
    ~11.5us/band stream. The STT op pays a dual-SBUF-read port penalty
    (1.22us vs 0.69us tensor_scalar) but every attempt to offload slabs
    to ACT scale-casts (interleaved, adds-last, or casts-pipelined-one-
    band-ahead) measured 4-20us SLOWER: Bacc's single-semaphore wait
    merging makes DVE adds transitively wait on unrelated earlier ACT
    queue entries (prior-band staging), and GpSimd/Pool elementwise is
    ~15us/slab (Q7 software path; TensorScalarPtr is ISA-rejected on
    Pool; PSUM APs are ISA-rejected on TensorScalarPtr too).
  - x is loaded f32 on the sync ring AFTER band 0's first two groups:
    its scatter layout ([p, chunk, d], m = c*128+p on partitions) needs
    1024 tiny 256 B descriptors with ~40 ns/descriptor overhead (~6 us
    of SDMA time) that must not sit in front of the chain-init slab.
    One ACT op casts it to fp16. (gpsimd SWDGE cast-during-DMA emitted
    1040 software packets + a 17.7 us ring drain - dead end.)
  - TensorE transposes the summed kernel (8 fp16 [128,128] tiles/band,
    ~110 ns cadence pipelined) via a GPSIMD-built identity, ACT stages
    PSUM->SBUF (the last band splits staging across ACT and DVE to
    halve the end-of-kernel serial tail), and the contraction accumulates
    in PSUM over 8 chunks. The tail stage is emitted one iteration late
    (software pipelining) so no cross-band serialized loop forms.
  - The 1/sum(coefs) normalization rides free on the ScalarE PSUM->SBUF
    output copy (activation scale); per-band output DMAs go on the ACT
    ring so they never block the sync-ring load stream. fp16 keeps the
    total quantization error ~5e-4 << the 2e-2 gate.
"""

import sys

if "/opt/trn_rl_repo" not in sys.path:
    sys.path.insert(0, "/opt/trn_rl_repo")

import numpy as np

import concourse.bass as bass
import concourse.mybir as mybir
from concourse import tile
from concourse.bacc import Bacc
from concourse.masks import make_identity
from concourse.bass_utils import run_bass_kernel_spmd

# Problem shapes (hardcoded per the harness contract).
B, K, N, D = 8, 9, 1024, 64
EPS = 0.01
P = 128          # SBUF partitions
NT = N // P      # 8 row/col tiles of the [N, N] kernel

F32 = mybir.dt.float32
F16 = mybir.dt.float16

R = 1.0 - EPS                                  # coefficient ratio
S = float(sum(R ** k for k in range(K)))       # sum of coefs

# Slab groups per band-DMA, in issue order. The chain consumes groups in
# this order, so put the high-k slabs (chain start) in the first DMA.
# The LAST band gets its low-k group split once more so the final
# serial tail waits on a single 0.5 MB slab instead of 1.5 MB.
GROUPS = [(6, 9), (3, 6), (0, 3)]
GROUPS_LAST = [(6, 9), (3, 6), (1, 3), (0, 1)]

# Who handles each slab. Measured costs (full [128,1024] slab):
#   ACT activation-scale f32->f16: 1.13us
#   DVE tensor_scalar cast 0.69us / fp16 tensor_tensor add 0.68us /
#       fused scalar_tensor_tensor (wk = g*c + wk) 1.22us
#   Pool TENSOR_SCALAR measured 14.9us (!) - Q7 software path, unusable.
# ACT casts {7,5,3,1} into gh tiles (DVE then adds them, 0.68 each);
# DVE takes {6,4,2,0} as fused STT links directly on the chain and
# inits the chain with k=8. Per band: DVE ~8.3us, ACT ~8.5us, both
# ~3us under the ~11.5us stream cadence.
ACT_CASTS = {7, 1}


def build_bass() -> bass.Bass:
    # Bacc (not plain Bass): its compile() runs generate_event_semaphores /
    # move_matmul_waits_to_ldweights, splitting multi-semaphore waits that
    # the 64B ISA instructions (single EVENTS slot) cannot carry.
    nc = Bacc()

    x_d = nc.dram_tensor("x_b", (N, D), F32, kind="ExternalInput")
    g_d = nc.dram_tensor("g_b", (K, N, N), F32, kind="ExternalInput")
    o_d = nc.dram_tensor("out_b", (N, D), F32, kind="ExternalOutput")

    MULT = mybir.AluOpType.mult
    ADD = mybir.AluOpType.add

    with tile.TileContext(nc) as tc:
        with (
            tc.tile_pool(name="consts", bufs=1) as consts,
            tc.tile_pool(name="gr", bufs=3) as gr_pool,
            tc.tile_pool(name="grl", bufs=1) as grl_pool,
            tc.tile_pool(name="wk", bufs=4) as wk_pool,
            tc.tile_pool(name="gh", bufs=3) as gh_pool,
            tc.tile_pool(name="wkt", bufs=2) as wkt_pool,
            tc.tile_pool(name="outp", bufs=2) as out_pool,
            tc.tile_pool(name="ps_t", bufs=4, space=bass.MemorySpace.PSUM) as ps_t,
            tc.tile_pool(name="ps_e", bufs=2, space=bass.MemorySpace.PSUM) as ps_e,
            tc.tile_pool(name="ps_wk", bufs=2, space=bass.MemorySpace.PSUM) as ps_wk,
        ):
            g_r = g_d.rearrange("k (b p) n -> b p k n", p=P)

            def load_band(i):
                """Per-band HWDGE DMAs of slab groups; returns {k: slice}."""
                tiles = {}
                assert i >= 1
                groups = GROUPS_LAST if i == NT - 1 else GROUPS
                for gi, (k0, k1) in enumerate(groups):
                    # distinct tag per (slot, width) so tile shapes stay
                    # consistent within a tag across bands; the last band's
                    # split low-k groups are single-use (bufs=1 pool)
                    tag = f"g{gi}w{k1 - k0}"
                    pool = gr_pool if (k1 - k0) == 3 else grl_pool
                    t = pool.tile([P, (k1 - k0) * N], F32, tag=tag, name=tag)
                    nc.sync.dma_start(t[:], g_r[i, :, k0:k1, :])
                    for k in range(k0, k1):
                        tiles[k] = t[:, (k - k0) * N : (k - k0 + 1) * N]
                return tiles

            # x: f32 on the sync HWDGE ring, issued BEFORE the band loads.
            # Its scatter layout needs 1024 tiny 256 B descriptors; issued
            # first they drain on idle SDMA engines in ~1us. Issued behind
            # the slab flood (or on the ACT ring) the same transfer took
            # ~15us of round-robin interleave with the 4 KB stream packets
            # and stalled the first band's consumers. One ACT op then casts
            # f32 -> bf16. Layout [p, chunk, d]: chunk c is a [128, 64]
            # tile with the contraction index m = c*128 + p on partitions.
            # (NOT gpsimd SWDGE either: the software descriptor path
            # emitted 1040 packets + a 17.7us ring drain that pinned
            # GpSimd until ~27us.)
            # Band 0's k=8 (chain init) and k=6..7 slabs go FIRST on the
            # sync ring so the chain starts on a 0.5 MB arrival (~9.5us).
            # The x scatter-load (1024 tiny 256 B descriptors, ~40ns each
            # of SDMA time) follows them: issued at the very head it
            # delayed the init slab by ~6us (measured: chain init at
            # 16.5us instead of ~10).
            band_tiles = {}
            for k0, k1, tag in ((8, 9, "gs89"), (6, 8, "gs68")):
                t = grl_pool.tile([P, (k1 - k0) * N], F32, tag=tag, name=tag)
                nc.sync.dma_start(t[:], g_r[0, :, k0:k1, :])
                for k in range(k0, k1):
                    band_tiles[k] = t[:, (k - k0) * N : (k - k0 + 1) * N]

            x_f32 = consts.tile([P, NT, D], F32)
            nc.sync.dma_start(x_f32[:], x_d.rearrange("(c p) d -> p c d", p=P))
            x_sb = consts.tile([P, NT, D], F16)
            nc.scalar.activation(
                x_sb[:], x_f32[:], mybir.ActivationFunctionType.Copy, scale=1.0
            )

            for k0, k1, tag in ((3, 6, "g1w3"), (0, 3, "g2w3")):
                t = gr_pool.tile([P, (k1 - k0) * N], F32, tag=tag, name=tag)
                nc.sync.dma_start(t[:], g_r[0, :, k0:k1, :])
                for k in range(k0, k1):
                    band_tiles[k] = t[:, (k - k0) * N : (k - k0 + 1) * N]

            # fp16 identity for TensorE transpose. Built by GPSIMD (memset +
            # affine_select, ~0.5us), then copied through VectorE so the
            # first PE transpose waits on a single semaphore (DVE) -
            # Matmult lowering only supports one sync wait.
            ident_raw = consts.tile([P, P], F16)
            make_identity(nc, ident_raw)
            ident = consts.tile([P, P], F16)
            nc.vector.tensor_copy(ident[:], ident_raw[:])

            # Post-k-sum pipeline for one band: transposes, wkT staging,
            # contraction, output. Emitted one iteration LATE (software
            # pipelining) so no cross-band serialized loop forms.
            def emit_tail(i, wk):
                wkT_sb = wkt_pool.tile([P, NT, P], F16, name="wkT_sb")
                for c in range(NT):
                    wkT_ps = ps_t.tile([P, P], F16, name="wkT_ps")
                    nc.tensor.transpose(wkT_ps[:], wk[:, c * P : (c + 1) * P], ident[:])
                    if i == NT - 1 and c % 2 == 1:
                        # last band's staging IS the serial tail: odd
                        # chunks ride DVE (idle after its chain) so both
                        # halves move in parallel.
                        nc.vector.tensor_copy(wkT_sb[:, c, :], wkT_ps[:])
                    else:
                        nc.scalar.copy(wkT_sb[:, c, :], wkT_ps[:])

                # emb[band i] = sum_c wk_tile(i,c) @ x_chunk(c), accumulated
                # in PSUM (fp32) over the 8 contraction chunks.
                emb_ps = ps_e.tile([P, D], F32, name="emb_ps")
                for c in range(NT):
                    nc.tensor.matmul(
                        emb_ps[:],
                        wkT_sb[:, c, :],
                        x_sb[:, c, :],
                        start=(c == 0),
                        stop=(c == NT - 1),
                    )

                # PSUM -> SBUF with the 1/S normalization folded into the
                # ACT copy's free scale.
                o_sb = out_pool.tile([P, D], F32, name="o_sb")
                nc.scalar.activation(
                    o_sb[:], emb_ps[:], mybir.ActivationFunctionType.Copy,
                    scale=1.0 / S,
                )
                # Output DMA on the ACT HWDGE ring, NOT sync: an out-DMA on
                # the sync FIFO would block the f32 slab loads queued behind
                # it until this band's whole pipeline finishes.
                nc.scalar.dma_start(o_d[i * P : (i + 1) * P, :], o_sb[:])

            pending = None
            for i in range(NT):
                g_ts = band_tiles
                if i + 1 < NT:
                    band_tiles = load_band(i + 1)

                # Weighted k-sum, wk = sum_k r^k * g_k, accumulated in fp16.
                # The scale+cast of each slab is spread across ACT / Pool /
                # DVE per the *_CASTS tables (the serial add chain stays on
                # DVE: fp16 tensor_tensor ADD, 0.68us). k=8 is the DVE chain
                # init; k=0 rides a fused DVE scalar_tensor_tensor at the
                # end (1.22us, cheaper than a separate cast+add there).
                # All-DVE serial chain, accumulator in SBUF. (PSUM
                # accumulator was tried to dodge the dual-SBUF-read port
                # penalty on scalar_tensor_tensor, but the core-v3 ISA
                # rejects PSUM APs on TensorScalarPtr.)
                wk = wk_pool.tile([P, N], F16, name="wk")
                nc.vector.tensor_scalar_mul(wk[:], g_ts[8], R ** 8)
                for k in (7, 6, 5, 4, 3, 2, 1, 0):
                    nc.vector.scalar_tensor_tensor(
                        wk[:], g_ts[k], R ** k, wk[:], MULT, ADD,
                    )

                if pending is not None:
                    emit_tail(*pending)
                pending = (i, wk)

            emit_tail(*pending)

    nc.compile()
    return nc


_NC = None


def _get_nc() -> bass.Bass:
    global _NC
    if _NC is None:
        _NC = build_bass()
    return _NC


def run(x: np.ndarray, gr_kernel: np.ndarray, **spmd_kwargs):
    """Run the SPMD kernel on cores 0-7; returns BassKernelResults."""
    nc = _get_nc()
    in_maps = [
        {
            "x_b": np.ascontiguousarray(x[b], dtype=np.float32),
            "g_b": np.ascontiguousarray(gr_kernel[b], dtype=np.float32),
        }
        for b in range(B)
    ]
    return run_bass_kernel_spmd(nc, in_maps, core_ids=list(range(B)), **spmd_kwargs)


def kernel(x: np.ndarray, gr_kernel: np.ndarray) -> np.ndarray:
    res = run(np.asarray(x), np.asarray(gr_kernel))
    out = np.stack([res.results[b]["out_b"] for b in range(B)], axis=0)
    return out.astype(np.float32, copy=False)


if __name__ == "__main__":
    rng = np.random.default_rng(0)
    x = rng.standard_normal((B, N, D), dtype=np.float32)
    g = rng.standard_normal((B, K, N, N), dtype=np.float32)
    out = kernel(x, g)
    coefs = (1.0 - EPS) ** np.arange(K)
    wk = np.einsum("k,bknm->bnm", coefs, g)
    ref = np.matmul(wk, x) / coefs.sum()
    err = np.linalg.norm(out - ref) / np.linalg.norm(ref)
    print("self-check rel err:", err)
